# revision 12
# baseline (speedup 1.0000x reference)
"""Distributed Trainium2 kernel for cross-attention (nn_Attention_50732153701013).

Reference computation (b=2, n=2048, dim=1024, heads=16, d_head=64):
    qkv  = split(x  @ W_qkv)          -> q,  k,  v
    qkv1 = split(x1 @ W_qkv)          -> q1, k1, v1
    out  = merge(softmax(q  k1^T / 8) v1) @ W_out + b_out
    out1 = merge(softmax(q1 k ^T / 8) v ) @ W_out + b_out

Sharding over 8 cores: core c handles batch (c // 4) and heads
[(c%4)*4, (c%4)*4+4).  Each core computes its 4 heads' attention for both
cross directions plus the partial out-projection (row-slice of W_out);
the host sums the 4 partial outputs per batch.

Device-side layout notes:
  * x/x1 are pre-transposed on the host to xT [dim, n] so every matmul
    contraction (over dim / d_head / n) has its axis on SBUF partitions.
  * Scores are computed transposed: S^T[m, n] = k[m]·q[n], so the softmax
    reduction axis (m) lies on PSUM partitions.  exp() needs no max
    subtraction (|scores| <~ 6).  The softmax denominator is obtained by
    appending a ones-column to V, so the AV matmul also yields
    colsum(exp S^T) as PSUM row 64.  The division happens on the 64-row
    O^T tile: reciprocal of the colsum row, broadcast across partitions
    with a PE outer product (ones[1,64]^T x recip[1,512]).
  * Matmul operands are bf16 (1 PE cycle/row + fast weight load; fp32 runs
    the HI/LO path at less than half throughput).  PSUM accumulation stays
    fp32.  Measured end-to-end relative error ~6e-3, gate is 2e-2.
  * The out-projection of block nb is emitted one attention unit late so
    its TensorE burst hides inside the Scalar engine's exp slack instead
    of stalling the softmax pipeline at block boundaries.
"""

import numpy as np

B, N, DIM = 2, 2048, 1024
HEADS, DHEAD = 16, 64
H_LOC = 4                 # heads per core
INNER_LOC = H_LOC * DHEAD  # 256
NCORES = 8
SCALE = DHEAD ** -0.5     # 0.125

_CACHED = {}


def _build_graph():
    import concourse.mybir as mybir
    from concourse import bacc
    from concourse.tile import TileContext

    f32 = mybir.dt.float32
    bf16 = mybir.dt.bfloat16
    AF = mybir.ActivationFunctionType

    nc = bacc.Bacc(None, target_bir_lowering=False)

    xT = nc.dram_tensor("xT", [DIM, N], f32, kind="ExternalInput")
    x1T = nc.dram_tensor("x1T", [DIM, N], f32, kind="ExternalInput")
    wqkv = nc.dram_tensor("wqkv", [DIM, 3 * INNER_LOC], f32, kind="ExternalInput")
    wout = nc.dram_tensor("wout", [INNER_LOC, DIM], f32, kind="ExternalInput")
    out = nc.dram_tensor("out", [2, N, DIM], f32, kind="ExternalOutput")

    KO = DIM // 128            # 8 contraction chunks for the projections
    NB = 4                     # n blocks of 512
    NT = N // 128              # 16 n tiles / m chunks
    VW = DHEAD + 1             # 65: head slice width in v_sb (data + ones col)

    with TileContext(nc) as tc:
        with (
            nc.allow_low_precision(reason="bf16 matmul operands, fp32 accum"),
            tc.tile_pool(name="persist", bufs=1) as persist,
            tc.tile_pool(name="qk", bufs=1) as qkpool,
        ):
            wqkv_sb = persist.tile([128, KO, 3 * INNER_LOC], bf16)
            nc.gpsimd.dma_start(
                wqkv_sb[:], wqkv.rearrange("(ko p) c -> p ko c", p=128)
            )
            wout_sb = persist.tile([128, 2, DIM], bf16)
            nc.gpsimd.dma_start(
                wout_sb[:], wout.rearrange("(ki p) d -> p ki d", p=128)
            )
            ones_f32 = persist.tile([128, 1], f32)
            nc.any.memset(ones_f32[:], 1.0)
            ones_row = persist.tile([1, 64], bf16)
            nc.vector.tensor_copy(
                ones_row[:], ones_f32[0:1, :].broadcast_to([1, 64])
            )

            # transposed q/k for both inputs: [128, chunk(2), n]
            qT = qkpool.tile([128, 2, N], bf16, tag="qT")
            kT = qkpool.tile([128, 2, N], bf16, tag="kT")
            q1T = qkpool.tile([128, 2, N], bf16, tag="q1T")
            k1T = qkpool.tile([128, 2, N], bf16, tag="k1T")
            # v in [m, head-slices] layout, ones col per head at offset 64
            v_sb = persist.tile([128, NT, H_LOC * VW], bf16, tag="v")
            v1_sb = persist.tile([128, NT, H_LOC * VW], bf16, tag="v1")
            for vt in (v_sb, v1_sb):
                nc.vector.tensor_copy(
                    vt[:].rearrange("p t (h c) -> p t h c", h=H_LOC)[:, :, :, DHEAD:],
                    ones_f32[:, None, None, :].broadcast_to([128, NT, H_LOC, 1]),
                )

            # ---------------- Stage 1: QKV projections ----------------
            with (
                tc.tile_pool(name="xstage", bufs=2) as xstage,
                tc.tile_pool(name="ps_qk", bufs=4, space="PSUM") as ps_qk,
                tc.tile_pool(name="ps_v", bufs=4, space="PSUM") as ps_v,
            ):
                for src_i, (srcT, qdst, kdst, vdst) in enumerate(
                    ((xT, qT, kT, v_sb), (x1T, q1T, k1T, v1_sb))
                ):
                    for half in range(2):
                        nslc = slice(half * 1024, (half + 1) * 1024)
                        xs = xstage.tile([128, KO, 1024], bf16, tag="xs")
                        nc.gpsimd.dma_start(
                            xs[:],
                            srcT.rearrange("(ko p) n -> p ko n", p=128)[:, :, nslc],
                        )
                        # q and k chunks ([128, 512] psum, accumulate over ko)
                        for mb in range(4):  # 0,1 -> q chunks; 2,3 -> k chunks
                            dst = qdst if mb < 2 else kdst
                            ci = mb % 2
                            for nb in range(2):
                                ps = ps_qk.tile([128, 512], f32, tag="ps_qk")
                                for ko in range(KO):
                                    nc.tensor.matmul(
                                        ps[:],
                                        wqkv_sb[:, ko, mb * 128:(mb + 1) * 128],
                                        xs[:, ko, nb * 512:(nb + 1) * 512],
                                        start=(ko == 0),
                                        stop=(ko == KO - 1),
                                    )
                                nc.vector.tensor_copy(
                                    dst[:, ci,
                                        half * 1024 + nb * 512:
                                        half * 1024 + (nb + 1) * 512],
                                    ps[:],
                                )
                        # v tiles ([n_tile 128, 256] psum)
                        for nt in range(8):
                            nt_g = half * 8 + nt
                            ps = ps_v.tile([128, INNER_LOC], f32, tag="ps_v")
                            for ko in range(KO):
                                nc.tensor.matmul(
                                    ps[:],
                                    xs[:, ko, nt * 128:(nt + 1) * 128],
                                    wqkv_sb[:, ko, 2 * INNER_LOC:3 * INNER_LOC],
                                    start=(ko == 0),
                                    stop=(ko == KO - 1),
                                )
                            nc.vector.tensor_copy(
                                vdst[:, nt_g, :]
                                .rearrange("p (h c) -> p h c", h=H_LOC)[:, :, :DHEAD],
                                ps[:].rearrange("p (h c) -> p h c", h=H_LOC),
                            )

            # ---------------- Stage 2: attention + out-projection ----------------
            with (
                tc.tile_pool(name="attn", bufs=4) as attn,
                tc.tile_pool(name="otp", bufs=5) as otp,
                tc.tile_pool(name="outstage", bufs=3) as outstage,
                tc.tile_pool(name="ps_s", bufs=2, space="PSUM") as ps_s,
                tc.tile_pool(name="ps_o", bufs=2, space="PSUM") as ps_o,
                tc.tile_pool(name="ps_out", bufs=1, space="PSUM") as ps_out,
                tc.tile_pool(name="ps_pb", bufs=1, space="PSUM") as ps_pb,
            ):
                ots = {}

                def attention(nb, d):
                    nslc = slice(nb * 512, (nb + 1) * 512)
                    qsrc, ksrc, vsrc = (
                        (qT, k1T, v1_sb) if d == 0 else (q1T, kT, v_sb)
                    )
                    ot = otp.tile([128, 2, 512], bf16, tag="ot")
                    ots[(nb, d)] = ot
                    # heads processed in pairs: even head on PE rows 0-63,
                    # odd head on rows 64-127 -> score matmuls pack on the
                    # array and the exp pipeline never drains inside a pair.
                    for hp in range(H_LOC // 2):
                        chunk = hp
                        po_a = ps_o.tile([128, 512], f32, tag="po")
                        po_b = ps_o.tile([128, 512], f32, tag="po")
                        pos = [po_a, po_b]
                        for mcp in range(8):
                            for sub, h in enumerate((2 * hp, 2 * hp + 1)):
                                prow = slice(sub * 64, sub * 64 + 64)
                                ps = ps_s.tile([128, 1024], f32, tag="ps_s")
                                for j in range(2):
                                    mc = mcp * 2 + j
                                    nc.tensor.matmul(
                                        ps[:, j * 512:(j + 1) * 512],
                                        ksrc[prow, chunk,
                                             mc * 128:(mc + 1) * 128],
                                        qsrc[prow, chunk, nslc],
                                        start=True,
                                        stop=True,
                                    )
                                a = attn.tile([128, 1024], bf16, tag="a")
                                nc.scalar.activation(a[:], ps[:], AF.Exp,
                                                     scale=SCALE)
                                for j in range(2):
                                    mc = mcp * 2 + j
                                    nc.tensor.matmul(
                                        pos[sub][0:VW, :],
                                        vsrc[:, mc, h * VW:(h + 1) * VW],
                                        a[:, j * 512:(j + 1) * 512],
                                        start=(mc == 0),
                                        stop=(mc == NT - 1),
                                    )
                        for sub, h in enumerate((2 * hp, 2 * hp + 1)):
                            prow = slice(sub * 64, sub * 64 + 64)
                            po = pos[sub]
                            csrow = attn.tile([1, 512], f32, tag="csrow")
                            nc.vector.tensor_copy(csrow[:], po[64:65, :])
                            recip_f = attn.tile([1, 512], f32, tag="recip_f")
                            nc.vector.reciprocal_approx_fast(
                                out=recip_f[:], in_=csrow[:]
                            )
                            recip = attn.tile([1, 512], bf16, tag="recip")
                            nc.vector.tensor_copy(recip[:], recip_f[:])
                            pb = ps_pb.tile([64, 512], f32, tag="ps_pb")
                            nc.tensor.matmul(pb[:], ones_row[:], recip[:],
                                             start=True, stop=True)
                            nc.vector.tensor_copy(ot[prow, chunk, :],
                                                  po[0:64, :])
                            nc.vector.tensor_mul(
                                ot[prow, chunk, :], ot[prow, chunk, :], pb[:]
                            )

                def proj(nb, dirs=(0, 1)):
                    for d in dirs:
                        for nt in range(4):
                            ob = outstage.tile([128, DIM], f32, tag="ob")
                            for db in range(2):
                                ps = ps_out.tile([128, 512], f32, tag="ps_out")
                                for ki in range(2):
                                    nc.tensor.matmul(
                                        ps[:],
                                        ots[(nb, d)][:, ki,
                                                     nt * 128:(nt + 1) * 128],
                                        wout_sb[:, ki, db * 512:(db + 1) * 512],
                                        start=(ki == 0),
                                        stop=(ki == 1),
                                    )
                                nc.vector.tensor_copy(
                                    ob[:, db * 512:(db + 1) * 512], ps[:]
                                )
                            nc.sync.dma_start(
                                out[d, nb * 512 + nt * 128:
                                    nb * 512 + (nt + 1) * 128, :],
                                ob[:],
                            )
                    for d in dirs:
                        del ots[(nb, d)]

                # proj(nb) is emitted one attention unit late so its PE burst
                # overlaps exp work instead of stalling it.
                attention(0, 0)
                attention(0, 1)
                attention(1, 0)
                proj(0)
                attention(1, 1)
                attention(2, 0)
                proj(1)
                attention(2, 1)
                attention(3, 0)
                proj(2)
                proj(3, dirs=(0,))
                attention(3, 1)
                proj(3, dirs=(1,))
    return nc


def _get_graph():
    if "nc" not in _CACHED:
        nc = _build_graph()
        # Bacc defers register allocation to finalize(); the pjrt exec path
        # serializes nc.m directly, so finalize here.
        nc.finalize()
        _CACHED["nc"] = nc
    return _CACHED["nc"]


def _make_in_maps(x, x1, W_qkv, W_out):
    in_maps = []
    for c in range(NCORES):
        b = c // 4
        h0 = (c % 4) * H_LOC
        cols = np.concatenate(
            [W_qkv[:, j * DIM + h0 * DHEAD: j * DIM + (h0 + H_LOC) * DHEAD]
             for j in range(3)],
            axis=1,
        )
        in_maps.append({
            "xT": np.ascontiguousarray(x[b].T).astype(np.float32, copy=False),
            "x1T": np.ascontiguousarray(x1[b].T).astype(np.float32, copy=False),
            "wqkv": np.ascontiguousarray(cols).astype(np.float32, copy=False),
            "wout": np.ascontiguousarray(
                W_out[h0 * DHEAD:(h0 + H_LOC) * DHEAD, :]
            ).astype(np.float32, copy=False),
        })
    return in_maps


def _run(x, x1, W_qkv, W_out, b_out, **spmd_kwargs):
    from concourse.bass_utils import run_bass_kernel_spmd

    nc = _get_graph()
    in_maps = _make_in_maps(x, x1, W_qkv, W_out)
    res = run_bass_kernel_spmd(nc, in_maps, core_ids=list(range(NCORES)),
                               **spmd_kwargs)
    parts = [r["out"].reshape(2, N, DIM) for r in res.results]
    out = np.zeros((B, N, DIM), np.float32)
    out1 = np.zeros((B, N, DIM), np.float32)
    for b in range(B):
        grp = parts[4 * b:4 * b + 4]
        out[b] = sum(p[0] for p in grp) + b_out
        out1[b] = sum(p[1] for p in grp) + b_out
    return (out, out1), res


def kernel(x, x1, W_qkv, W_out, b_out):
    x = np.asarray(x, np.float32)
    x1 = np.asarray(x1, np.float32)
    W_qkv = np.asarray(W_qkv, np.float32)
    W_out = np.asarray(W_out, np.float32)
    b_out = np.asarray(b_out, np.float32)
    (out, out1), _ = _run(x, x1, W_qkv, W_out, b_out)
    return out, out1


# revision 13
# speedup vs baseline: 1.0237x; 1.0237x over previous
"""Distributed Trainium2 kernel for cross-attention (nn_Attention_50732153701013).

Reference computation (b=2, n=2048, dim=1024, heads=16, d_head=64):
    qkv  = split(x  @ W_qkv)          -> q,  k,  v
    qkv1 = split(x1 @ W_qkv)          -> q1, k1, v1
    out  = merge(softmax(q  k1^T / 8) v1) @ W_out + b_out
    out1 = merge(softmax(q1 k ^T / 8) v ) @ W_out + b_out

Sharding over 8 cores: core c handles batch (c // 4) and heads
[(c%4)*4, (c%4)*4+4).  Each core computes its 4 heads' attention for both
cross directions plus the partial out-projection (row-slice of W_out);
the host sums the 4 partial outputs per batch.

Device-side layout notes:
  * x/x1 are pre-transposed on the host to xT [dim, n] so every matmul
    contraction (over dim / d_head / n) has its axis on SBUF partitions.
  * Scores are computed transposed: S^T[m, n] = k[m]·q[n], so the softmax
    reduction axis (m) lies on PSUM partitions.  exp() needs no max
    subtraction (|scores| <~ 6).  The softmax denominator is obtained by
    appending a ones-column to V, so the AV matmul also yields
    colsum(exp S^T) as PSUM row 64.  The division happens on the 64-row
    O^T tile: reciprocal of the colsum row, broadcast across partitions
    with a PE outer product (ones[1,64]^T x recip[1,512]).
  * Matmul operands are bf16 (1 PE cycle/row + fast weight load; fp32 runs
    the HI/LO path at less than half throughput).  PSUM accumulation stays
    fp32.  Measured end-to-end relative error ~6e-3, gate is 2e-2.
  * The out-projection of block nb is emitted one attention unit late so
    its TensorE burst hides inside the Scalar engine's exp slack instead
    of stalling the softmax pipeline at block boundaries.
"""

import numpy as np

B, N, DIM = 2, 2048, 1024
HEADS, DHEAD = 16, 64
H_LOC = 4                 # heads per core
INNER_LOC = H_LOC * DHEAD  # 256
NCORES = 8
SCALE = DHEAD ** -0.5     # 0.125

_CACHED = {}


def _build_graph():
    import concourse.mybir as mybir
    from concourse import bacc
    from concourse.tile import TileContext

    f32 = mybir.dt.float32
    bf16 = mybir.dt.bfloat16
    AF = mybir.ActivationFunctionType

    nc = bacc.Bacc(None, target_bir_lowering=False)

    xT = nc.dram_tensor("xT", [DIM, N], f32, kind="ExternalInput")
    x1T = nc.dram_tensor("x1T", [DIM, N], f32, kind="ExternalInput")
    wqkv = nc.dram_tensor("wqkv", [DIM, 3 * INNER_LOC], f32, kind="ExternalInput")
    wout = nc.dram_tensor("wout", [INNER_LOC, DIM], f32, kind="ExternalInput")
    out = nc.dram_tensor("out", [2, N, DIM], f32, kind="ExternalOutput")

    KO = DIM // 128            # 8 contraction chunks for the projections
    NB = 4                     # n blocks of 512
    NT = N // 128              # 16 n tiles / m chunks
    VW = DHEAD + 1             # 65: head slice width in v_sb (data + ones col)

    with TileContext(nc) as tc:
        with (
            nc.allow_low_precision(reason="bf16 matmul operands, fp32 accum"),
            tc.tile_pool(name="persist", bufs=1) as persist,
            tc.tile_pool(name="qk", bufs=1) as qkpool,
        ):
            wqkv_sb = persist.tile([128, KO, 3 * INNER_LOC], bf16)
            nc.gpsimd.dma_start(
                wqkv_sb[:], wqkv.rearrange("(ko p) c -> p ko c", p=128)
            )
            wout_sb = persist.tile([128, 2, DIM], bf16)
            nc.gpsimd.dma_start(
                wout_sb[:], wout.rearrange("(ki p) d -> p ki d", p=128)
            )
            ones_f32 = persist.tile([128, 1], f32)
            nc.any.memset(ones_f32[:], 1.0)
            ones_row = persist.tile([1, 64], bf16)
            nc.vector.tensor_copy(
                ones_row[:], ones_f32[0:1, :].broadcast_to([1, 64])
            )

            # transposed q/k for both inputs: [128, chunk(2), n]
            qT = qkpool.tile([128, 2, N], bf16, tag="qT")
            kT = qkpool.tile([128, 2, N], bf16, tag="kT")
            q1T = qkpool.tile([128, 2, N], bf16, tag="q1T")
            k1T = qkpool.tile([128, 2, N], bf16, tag="k1T")
            # v in [m, head-slices] layout, ones col per head at offset 64
            v_sb = persist.tile([128, NT, H_LOC * VW], bf16, tag="v")
            v1_sb = persist.tile([128, NT, H_LOC * VW], bf16, tag="v1")
            for vt in (v_sb, v1_sb):
                nc.vector.tensor_copy(
                    vt[:].rearrange("p t (h c) -> p t h c", h=H_LOC)[:, :, :, DHEAD:],
                    ones_f32[:, None, None, :].broadcast_to([128, NT, H_LOC, 1]),
                )

            # ---------------- Stage 1: QKV projections ----------------
            with (
                tc.tile_pool(name="xstage", bufs=2) as xstage,
                tc.tile_pool(name="ps_qk", bufs=4, space="PSUM") as ps_qk,
                tc.tile_pool(name="ps_v", bufs=4, space="PSUM") as ps_v,
            ):
                for src_i, (srcT, qdst, kdst, vdst) in enumerate(
                    ((xT, qT, kT, v_sb), (x1T, q1T, k1T, v1_sb))
                ):
                    for half in range(2):
                        nslc = slice(half * 1024, (half + 1) * 1024)
                        xs = xstage.tile([128, KO, 1024], bf16, tag="xs")
                        nc.gpsimd.dma_start(
                            xs[:],
                            srcT.rearrange("(ko p) n -> p ko n", p=128)[:, :, nslc],
                        )
                        # q and k chunks ([128, 512] psum, accumulate over ko)
                        for mb in range(4):  # 0,1 -> q chunks; 2,3 -> k chunks
                            dst = qdst if mb < 2 else kdst
                            ci = mb % 2
                            for nb in range(2):
                                ps = ps_qk.tile([128, 512], f32, tag="ps_qk")
                                for ko in range(KO):
                                    nc.tensor.matmul(
                                        ps[:],
                                        wqkv_sb[:, ko, mb * 128:(mb + 1) * 128],
                                        xs[:, ko, nb * 512:(nb + 1) * 512],
                                        start=(ko == 0),
                                        stop=(ko == KO - 1),
                                    )
                                nc.vector.tensor_copy(
                                    dst[:, ci,
                                        half * 1024 + nb * 512:
                                        half * 1024 + (nb + 1) * 512],
                                    ps[:],
                                )
                        # v tiles ([n_tile 128, 256] psum)
                        for nt in range(8):
                            nt_g = half * 8 + nt
                            ps = ps_v.tile([128, INNER_LOC], f32, tag="ps_v")
                            for ko in range(KO):
                                nc.tensor.matmul(
                                    ps[:],
                                    xs[:, ko, nt * 128:(nt + 1) * 128],
                                    wqkv_sb[:, ko, 2 * INNER_LOC:3 * INNER_LOC],
                                    start=(ko == 0),
                                    stop=(ko == KO - 1),
                                )
                            nc.vector.tensor_copy(
                                vdst[:, nt_g, :]
                                .rearrange("p (h c) -> p h c", h=H_LOC)[:, :, :DHEAD],
                                ps[:].rearrange("p (h c) -> p h c", h=H_LOC),
                            )

            # ---------------- Stage 2: attention + out-projection ----------------
            with (
                tc.tile_pool(name="attn", bufs=4) as attn,
                tc.tile_pool(name="otp", bufs=5) as otp,
                tc.tile_pool(name="outstage", bufs=3) as outstage,
                tc.tile_pool(name="ps_s", bufs=2, space="PSUM") as ps_s,
                tc.tile_pool(name="ps_o", bufs=2, space="PSUM") as ps_o,
                tc.tile_pool(name="ps_out", bufs=1, space="PSUM") as ps_out,
                tc.tile_pool(name="ps_pb", bufs=1, space="PSUM") as ps_pb,
            ):
                ots = {}

                def attention(nb, d):
                    nslc = slice(nb * 512, (nb + 1) * 512)
                    qsrc, ksrc, vsrc = (
                        (qT, k1T, v1_sb) if d == 0 else (q1T, kT, v_sb)
                    )
                    ot = otp.tile([128, 2, 512], bf16, tag="ot")
                    ots[(nb, d)] = ot
                    for h in range(H_LOC):
                        prow = slice((h % 2) * 64, (h % 2) * 64 + 64)
                        chunk = h // 2
                        po = ps_o.tile([128, 512], f32, tag="po")
                        for mcp in range(8):
                            ps = ps_s.tile([128, 1024], f32, tag="ps_s")
                            for j in range(2):
                                mc = mcp * 2 + j
                                nc.tensor.matmul(
                                    ps[:, j * 512:(j + 1) * 512],
                                    ksrc[prow, chunk, mc * 128:(mc + 1) * 128],
                                    qsrc[prow, chunk, nslc],
                                    start=True,
                                    stop=True,
                                )
                            a = attn.tile([128, 1024], bf16, tag="a")
                            nc.scalar.activation(a[:], ps[:], AF.Exp, scale=SCALE)
                            for j in range(2):
                                mc = mcp * 2 + j
                                nc.tensor.matmul(
                                    po[0:VW, :],
                                    vsrc[:, mc, h * VW:(h + 1) * VW],
                                    a[:, j * 512:(j + 1) * 512],
                                    start=(mc == 0),
                                    stop=(mc == NT - 1),
                                )
                        csrow = attn.tile([1, 512], f32, tag="csrow")
                        nc.vector.tensor_copy(csrow[:], po[64:65, :])
                        recip_f = attn.tile([1, 512], f32, tag="recip_f")
                        nc.vector.reciprocal_approx_fast(
                            out=recip_f[:], in_=csrow[:]
                        )
                        recip = attn.tile([1, 512], bf16, tag="recip")
                        nc.vector.tensor_copy(recip[:], recip_f[:])
                        pb = ps_pb.tile([64, 512], f32, tag="ps_pb")
                        nc.tensor.matmul(pb[:], ones_row[:], recip[:],
                                         start=True, stop=True)
                        nc.vector.tensor_copy(ot[prow, chunk, :], po[0:64, :])
                        nc.vector.tensor_mul(
                            ot[prow, chunk, :], ot[prow, chunk, :], pb[:]
                        )

                def proj(nb, dirs=(0, 1)):
                    for d in dirs:
                        for nt in range(4):
                            ob = outstage.tile([128, DIM], f32, tag="ob")
                            for db in range(2):
                                ps = ps_out.tile([128, 512], f32, tag="ps_out")
                                for ki in range(2):
                                    nc.tensor.matmul(
                                        ps[:],
                                        ots[(nb, d)][:, ki,
                                                     nt * 128:(nt + 1) * 128],
                                        wout_sb[:, ki, db * 512:(db + 1) * 512],
                                        start=(ki == 0),
                                        stop=(ki == 1),
                                    )
                                nc.vector.tensor_copy(
                                    ob[:, db * 512:(db + 1) * 512], ps[:]
                                )
                            nc.sync.dma_start(
                                out[d, nb * 512 + nt * 128:
                                    nb * 512 + (nt + 1) * 128, :],
                                ob[:],
                            )
                    for d in dirs:
                        del ots[(nb, d)]

                # proj(nb) is emitted one attention unit late so its PE burst
                # overlaps exp work instead of stalling it.
                attention(0, 0)
                attention(0, 1)
                attention(1, 0)
                proj(0)
                attention(1, 1)
                attention(2, 0)
                proj(1)
                attention(2, 1)
                attention(3, 0)
                proj(2)
                proj(3, dirs=(0,))
                attention(3, 1)
                proj(3, dirs=(1,))
    return nc


def _get_graph():
    if "nc" not in _CACHED:
        nc = _build_graph()
        # Bacc defers register allocation to finalize(); the pjrt exec path
        # serializes nc.m directly, so finalize here.
        nc.finalize()
        _CACHED["nc"] = nc
    return _CACHED["nc"]


def _make_in_maps(x, x1, W_qkv, W_out):
    in_maps = []
    for c in range(NCORES):
        b = c // 4
        h0 = (c % 4) * H_LOC
        cols = np.concatenate(
            [W_qkv[:, j * DIM + h0 * DHEAD: j * DIM + (h0 + H_LOC) * DHEAD]
             for j in range(3)],
            axis=1,
        )
        in_maps.append({
            "xT": np.ascontiguousarray(x[b].T).astype(np.float32, copy=False),
            "x1T": np.ascontiguousarray(x1[b].T).astype(np.float32, copy=False),
            "wqkv": np.ascontiguousarray(cols).astype(np.float32, copy=False),
            "wout": np.ascontiguousarray(
                W_out[h0 * DHEAD:(h0 + H_LOC) * DHEAD, :]
            ).astype(np.float32, copy=False),
        })
    return in_maps


def _run(x, x1, W_qkv, W_out, b_out, **spmd_kwargs):
    from concourse.bass_utils import run_bass_kernel_spmd

    nc = _get_graph()
    in_maps = _make_in_maps(x, x1, W_qkv, W_out)
    res = run_bass_kernel_spmd(nc, in_maps, core_ids=list(range(NCORES)),
                               **spmd_kwargs)
    parts = [r["out"].reshape(2, N, DIM) for r in res.results]
    out = np.zeros((B, N, DIM), np.float32)
    out1 = np.zeros((B, N, DIM), np.float32)
    for b in range(B):
        grp = parts[4 * b:4 * b + 4]
        out[b] = sum(p[0] for p in grp) + b_out
        out1[b] = sum(p[1] for p in grp) + b_out
    return (out, out1), res


def kernel(x, x1, W_qkv, W_out, b_out):
    x = np.asarray(x, np.float32)
    x1 = np.asarray(x1, np.float32)
    W_qkv = np.asarray(W_qkv, np.float32)
    W_out = np.asarray(W_out, np.float32)
    b_out = np.asarray(b_out, np.float32)
    (out, out1), _ = _run(x, x1, W_qkv, W_out, b_out)
    return out, out1
